# revision 14
# baseline (speedup 1.0000x reference)
"""ConvCNP encoder kernel for 8 Trainium2 NeuronCores.

Computes, for full inputs X(4,1024,2), Y(4,1024,2), grid(16384,2):
    Gram = exp(-0.5*||grid-X||^2)          (B, G, n)
    FM   = Gram @ [1, Y]                   (B, G, 3)
    out  = [FM0, FM1/FM0, FM2/FM0] -> (B, 3, 128, 128)  (y, x image axes)

The reference grid is a meshgrid, so the 2-D RBF factors into 1-D
Gaussians: Gram[(x,y), n] = A1[x, n] * A2[y, n].

Sharding (v2): core = (batch b, y-half h).  Each core computes its
batch over y in [64h, 64h+64) and ALL 128 x columns:
  exp work/core = N*(64+128) elems vs N*B*(128+16) for a grid-shard.

Per-core pipeline (2 chunks of 4 n-tiles):
  mm1 (PE):  A2pre[n,y] = s2[7,n].T @ g2[7,64],  A1pre[n,x] likewise
  exp (ACT): PSUM -> SBUF A2/A1 (f16)
  V   (DVE): V[n,c,y] = A2[n,y] * E[n,c]  (broadcast mul)
  mm2 (PE):  FM[x,(c,y)] += A1[n,x].T @ V[n,(c,y)]  over 8 n-tiles
  norm:      recip + muls, density copy on ACT
  DMA out:   one [128, 768B] contiguous store; host reassembles.
"""

import numpy as np

B = 4
N = 1024
G = 16384
NCORES = 8
NT = N // 128             # 8 context tiles
KS = 7                    # 1-D factorization rows
GS = G // NCORES
XCOLS = GS // 128

_CACHE = {}


# ---------------------------------------------------------------------------
# v2 separable kernel: core = (batch, y-half)
# ---------------------------------------------------------------------------

def _build_nc_v2():
    import concourse.bacc as bacc
    import concourse.mybir as mybir
    import concourse.tile as tile
    from contextlib import ExitStack

    f32 = mybir.dt.float32
    f16 = mybir.dt.float16

    nc = bacc.Bacc("TRN2", target_bir_lowering=False, debug=False,
                   num_devices=NCORES)
    # column layout A: [G2 64 | G1 128 | s2 nt0..3 512 | s1 nt0..3 512]
    # column layout B: [s2 nt4..7 512 | s1 nt4..7 512]
    insa_d = nc.dram_tensor("INSA", [KS, 1216], f16, kind="ExternalInput")
    insb_d = nc.dram_tensor("INSB", [KS, 1024], f16, kind="ExternalInput")
    # E for channels 1,2 pre-broadcast along y: [128, nt, 2, 64]
    ine_d = nc.dram_tensor("INE", [128, NT * 2 * 64], f16, kind="ExternalInput")
    out_d = nc.dram_tensor("OUT", [128, 3, 64], f16, kind="ExternalOutput")

    EXP = mybir.ActivationFunctionType.Exp

    CH = (4, 4)               # n-tile chunk sizes
    C0 = CH[0]

    with tile.TileContext(nc) as tc, ExitStack() as ctx:
        consts = ctx.enter_context(tc.tile_pool(name="consts", bufs=1))
        ps2_pool = ctx.enter_context(tc.tile_pool(name="ps2", bufs=1, space="PSUM"))
        ps1_pool = ctx.enter_context(tc.tile_pool(name="ps1", bufs=1, space="PSUM"))
        fm0_pool = ctx.enter_context(tc.tile_pool(name="fm0", bufs=1, space="PSUM"))
        fm12_pool = ctx.enter_context(tc.tile_pool(name="fm12", bufs=1, space="PSUM"))
        a2_pool = ctx.enter_context(tc.tile_pool(name="a2", bufs=1))
        a1_pool = ctx.enter_context(tc.tile_pool(name="a1", bufs=1))
        v_pool = ctx.enter_context(tc.tile_pool(name="v", bufs=1))
        small = ctx.enter_context(tc.tile_pool(name="small", bufs=2))
        outp = ctx.enter_context(tc.tile_pool(name="outp", bufs=1))

        insa = consts.tile([KS, 1216], f16)
        insb = consts.tile([KS, 1024], f16)
        e_sb = consts.tile([128, NT * 2 * 64], f16)
        # first-needed columns on sync; INSB first on gpsimd; fat E split
        # over both queues
        nc.sync.dma_start(out=insa, in_=insa_d[:, :])
        nc.gpsimd.dma_start(out=insb, in_=insb_d[:, :])
        nc.sync.dma_start(out=e_sb[:, 0:C0 * 128], in_=ine_d[:, 0:C0 * 128])
        nc.gpsimd.dma_start(out=e_sb[:, C0 * 128:], in_=ine_d[:, C0 * 128:])

        e_v = e_sb.rearrange("p (t c y) -> p t c y", t=NT, c=2)
        g2 = insa[:, 0:64]
        g1 = insa[:, 64:192]

        def s2_sl(nt):
            t, off = (insa, 192) if nt < 4 else (insb, -512)
            return t[:, off + 128 * nt: off + 128 * (nt + 1)]

        def s1_sl(nt):
            t, off = (insa, 704) if nt < 4 else (insb, 0)
            return t[:, off + 128 * nt: off + 128 * (nt + 1)]

        out_sb = outp.tile([128, 3, 64], f16)
        fm0 = fm0_pool.tile([128, 64], f32)
        fm12 = fm12_pool.tile([128, 2, 64], f32)

        a2t, a1t, vt = {}, {}, {}
        # per chunk: A2 matmuls, A1 matmuls, exp(A2), exp(A1), V — this
        # interleaving lets mm2 (gated on a1 of chunk 0) start early
        for ch in range(2):
            n0 = ch * C0
            ps2 = ps2_pool.tile([128, CH[ch], 64], f32, tag=f"ps2{ch}",
                                name=f"ps2_{ch}")
            for i in range(CH[ch]):
                nc.tensor.matmul(ps2[:, i, :], s2_sl(n0 + i), g2,
                                 start=True, stop=True)
            ps1 = ps1_pool.tile([128, CH[ch], 128], f32, tag=f"ps1{ch}",
                                name=f"ps1_{ch}")
            for i in range(CH[ch]):
                nc.tensor.matmul(ps1[:, i, :], s1_sl(n0 + i), g1,
                                 start=True, stop=True)
            a2 = a2_pool.tile([128, CH[ch], 64], f16, tag=f"a2{ch}",
                              name=f"a2_{ch}")
            nc.scalar.activation(out=a2, in_=ps2, func=EXP)
            a2t[ch] = a2
            a1 = a1_pool.tile([128, CH[ch], 128], f16, tag=f"a1{ch}",
                              name=f"a1_{ch}")
            nc.scalar.activation(out=a1, in_=ps1, func=EXP)
            a1t[ch] = a1
            v = v_pool.tile([128, CH[ch], 2, 64], f16, tag=f"v{ch}",
                            name=f"v_{ch}")
            a2_b = a2[:, :, None, :].to_broadcast([128, CH[ch], 2, 64])
            nc.vector.tensor_mul(v, a2_b, e_v[:, n0:n0 + CH[ch], :, :])
            vt[ch] = v

        # mm2, chunk-major so chunk 1's groups are all that trail the last
        # exp; fm0 (density) closes before fm12 so recip/copy overlap
        for ch in range(2):
            for i in range(CH[ch]):
                nc.tensor.matmul(fm0[:], a1t[ch][:, i, :], a2t[ch][:, i, :],
                                 start=(ch == 0 and i == 0),
                                 stop=(ch == 1 and i == CH[1] - 1))
            for i in range(CH[ch]):
                nc.tensor.matmul(fm12[:], a1t[ch][:, i, :], vt[ch][:, i, :, :],
                                 start=(ch == 0 and i == 0),
                                 stop=(ch == 1 and i == CH[1] - 1))

        recip = small.tile([128, 64], f32, tag="recip")
        nc.vector.reciprocal_approx_fast(out=recip, in_=fm0[:])
        nc.scalar.copy(out_sb[:, 0, :], fm0[:])
        rec_b = recip[:, None, :].to_broadcast([128, 2, 64])
        nc.vector.tensor_mul(out_sb[:, 1:3, :], fm12[:], rec_b)
        nc.sync.dma_start(out=out_d[:, :, :], in_=out_sb)

    nc.compile()
    return nc


def _sep_factors(gv, xc):
    """K=7 fp16 factorization of -0.5 (g - x)^2 along one dimension.
    xc: (N,) context coords, gv: (M,) grid coords.
    Returns A [7, N] stationary rows and Bm [7, M] moving rows."""
    f16 = np.float16

    def split(a):
        hi = a.astype(f16).astype(np.float32)
        lo = (a - hi).astype(f16).astype(np.float32)
        return hi, lo

    sx = -0.5 * xc * xc
    sg = -0.5 * gv * gv
    xh, xl = split(xc)
    gh, gl = split(gv)
    sxh, sxl = split(sx)
    sgh, sgl = split(sg)
    on = np.ones_like(xc)
    og = np.ones_like(gv)
    A = np.stack([xh, xl, xh, sxh, sxl, on, on], axis=0)
    Bm = np.stack([gh, gh, gl, og, og, sgh, sgl], axis=0)
    return A.astype(f16), Bm.astype(f16)


def _prepare_inputs_v2(X, Y, grid):
    f16 = np.float16
    X = np.asarray(X, np.float32)
    Y = np.asarray(Y, np.float32)
    grid = np.asarray(grid, np.float32)
    gxv = grid[::128, 0]
    gyv = grid[:128, 1]

    in_maps = []
    for c in range(NCORES):
        b, h = divmod(c, 2)
        A1s, G1 = _sep_factors(gxv, X[b, :, 0])                 # [7,1024],[7,128]
        A2s, G2 = _sep_factors(gyv[64 * h:64 * h + 64], X[b, :, 1])  # [7,1024],[7,64]
        s1 = A1s.reshape(KS, NT, 128)
        s2 = A2s.reshape(KS, NT, 128)
        insa = np.concatenate(
            [G2, G1, s2[:, 0:4].reshape(KS, 512), s1[:, 0:4].reshape(KS, 512)],
            axis=1)
        insb = np.concatenate(
            [s2[:, 4:8].reshape(KS, 512), s1[:, 4:8].reshape(KS, 512)], axis=1)
        # E channels 1,2 pre-broadcast along y: [128, nt, 2, 64]
        e12 = Y[b].reshape(NT, 128, 2).transpose(1, 0, 2).astype(f16)
        ine = np.broadcast_to(e12[:, :, :, None],
                              (128, NT, 2, 64)).reshape(128, NT * 2 * 64)
        in_maps.append({"INSA": np.ascontiguousarray(insa).astype(f16),
                        "INSB": np.ascontiguousarray(insb).astype(f16),
                        "INE": np.ascontiguousarray(ine).astype(f16)})
    return in_maps


def _grid_separable(grid):
    grid = np.asarray(grid)
    if grid.shape != (G, 2):
        return False
    gxv = grid[::128, 0]
    gyv = grid[:128, 1]
    return (np.array_equal(grid[:, 0], np.repeat(gxv, 128))
            and np.array_equal(grid[:, 1], np.tile(gyv, 128)))


# ---------------------------------------------------------------------------
# General fallback (grid not a meshgrid): grid-axis shard, K=10 bf16
# hi/lo-split factorization of -0.5*d^2.  Unused for the reference grid.
# ---------------------------------------------------------------------------

NTG = N // 128
JS = GS // 512
K = 10
A_W = B * 2 * 128
B_W = GS
E_W = B * NTG * 5
IN_W = A_W + B_W + E_W


def _build_nc_general():
    import concourse.bacc as bacc
    import concourse.mybir as mybir
    import concourse.tile as tile
    from contextlib import ExitStack

    f32 = mybir.dt.float32
    bf16 = mybir.dt.bfloat16

    nc = bacc.Bacc("TRN2", target_bir_lowering=False, debug=False,
                   num_devices=NCORES)
    in_d = nc.dram_tensor("IN", [128, IN_W], bf16, kind="ExternalInput")
    out_d = nc.dram_tensor("OUT", [B, 3, 128, XCOLS], f32, kind="ExternalOutput")

    EXP = mybir.ActivationFunctionType.Exp

    with tile.TileContext(nc) as tc, ExitStack() as ctx:
        consts = ctx.enter_context(tc.tile_pool(name="consts", bufs=1))
        gram_pool = ctx.enter_context(tc.tile_pool(name="gram", bufs=4))
        mm1a_pool = ctx.enter_context(tc.tile_pool(name="mm1a", bufs=1, space="PSUM"))
        mm1b_pool = ctx.enter_context(tc.tile_pool(name="mm1b", bufs=1, space="PSUM"))
        mm2_pool = ctx.enter_context(tc.tile_pool(name="mm2", bufs=1, space="PSUM"))
        small = ctx.enter_context(tc.tile_pool(name="small", bufs=4))
        outp = ctx.enter_context(tc.tile_pool(name="outp", bufs=1))

        a0_sb = consts.tile([128, 2 * 128], bf16)
        a123_sb = consts.tile([128, 3 * 2 * 128], bf16)
        b_t = [consts.tile([128, 512], bf16, name=f"bj{j}", tag=f"bj{j}")
               for j in range(JS)]
        e_sb = consts.tile([128, E_W], bf16)

        def in_col(c0, w):
            return in_d[:, c0:c0 + w]

        nc.sync.dma_start(out=a0_sb, in_=in_col(0, 256))
        nc.sync.dma_start(out=b_t[0], in_=in_col(256, 512))
        nc.gpsimd.dma_start(out=b_t[1], in_=in_col(768, 512))
        nc.sync.dma_start(out=a123_sb, in_=in_col(1280, 768))
        nc.gpsimd.dma_start(out=b_t[2], in_=in_col(2048, 512))
        nc.sync.dma_start(out=b_t[3], in_=in_col(2560, 512))
        nc.gpsimd.dma_start(out=e_sb, in_=in_col(3072, E_W))

        a0_v = a0_sb.rearrange("p (h m) -> p h m", h=2)
        a123_v = a123_sb.rearrange("p (b h m) -> p b h m", b=3, h=2)
        e_v = e_sb.rearrange("p (b t c) -> p b t c", b=B, t=NTG)

        def a_slice(b, row, h4):
            if b == 0:
                return a0_v[32 * row:32 * row + K, h4, :]
            return a123_v[32 * row:32 * row + K, b - 1, h4, :]

        def b_slice(j):
            return b_t[j]

        out_sb = outp.tile([128, B, 3, XCOLS], f32)
        grams = {}

        def emit_mm1_group(b, slots, ps):
            for i, s in enumerate(slots):
                j = s // 8
                nt = s % 8
                row = nt % 4
                lhsT = a_slice(b, row, nt // 4)
                rhs = b_slice(j)[32 * row:32 * row + K, :]
                nc.tensor.matmul(ps[:, i, :], lhsT, rhs,
                                 start=True, stop=True,
                                 tile_position=(32 * row, 0))

        def emit_mm1_exp(b, h, sizes, tags):
            gram = gram_pool.tile([128, 16, 512], bf16, tag="gram",
                                  name=f"gram{b}{h}")
            grams[(b, h)] = gram
            s0 = 0
            for gsz, sel in zip(sizes, tags):
                pool = (mm1a_pool, mm1b_pool)[sel]
                cap = (4, 3)[sel]
                ps = pool.tile([128, cap, 512], f32, tag=f"t{sel}",
                               name=f"ps{sel}")
                emit_mm1_group(b, [16 * h + s0 + i for i in range(gsz)], ps)
                nc.scalar.activation(out=gram[:, s0:s0 + gsz, :],
                                     in_=ps[:, 0:gsz, :], func=EXP)
                s0 += gsz

        def emit_mm2_j(b, j, gram, base):
            fm = grams[("fm", b)]
            for r in range(4):
                gsub = j * 4 + r
                for nt in range(NTG):
                    nc.tensor.matmul(
                        fm[:, gsub, :],
                        gram[:, base + nt, r * 128:(r + 1) * 128],
                        e_v[:, b, nt, :],
                        start=(nt == 0),
                        stop=(nt == NTG - 1),
                    )

        def emit_norm(b, sl, dma_engine):
            fm = grams[("fm", b)]
            w = sl.stop - sl.start
            fmc = small.tile([128, 8, 5], f32, tag="fmc")
            nc.vector.tensor_copy(fmc[:, 0:w, :], fm[:, sl, :])
            recip = small.tile([128, 8], f32, tag="recip")
            nc.vector.reciprocal(recip[:, 0:w], fmc[:, 0:w, 0])
            nc.vector.tensor_copy(out_sb[:, b, 0, sl], fmc[:, 0:w, 0])
            v1 = small.tile([128, 8], f32, tag="v1")
            nc.vector.tensor_add(v1[:, 0:w], fmc[:, 0:w, 1], fmc[:, 0:w, 3])
            nc.vector.tensor_mul(out_sb[:, b, 1, sl], v1[:, 0:w], recip[:, 0:w])
            v2 = small.tile([128, 8], f32, tag="v2")
            nc.vector.tensor_add(v2[:, 0:w], fmc[:, 0:w, 2], fmc[:, 0:w, 4])
            nc.vector.tensor_mul(out_sb[:, b, 2, sl], v2[:, 0:w], recip[:, 0:w])
            dst = out_d[b, :, :, sl].rearrange("c y x -> y c x")
            dma_engine.dma_start(out=dst, in_=out_sb[:, b, :, sl])

        half_patterns = [((2, 4, 3, 4, 3), (1, 0, 1, 0, 1))]
        for k in range(1, 7):
            if k % 2 == 1:
                half_patterns.append(((4, 3, 4, 3, 2), (0, 1, 0, 1, 0)))
            else:
                half_patterns.append(((3, 4, 3, 4, 2), (1, 0, 1, 0, 1)))

        for b in range(B):
            fm_t = mm2_pool.tile([128, XCOLS, 5], f32, tag="fm")
            grams[("fm", b)] = fm_t
            if b < B - 1:
                emit_mm1_exp(b, 0, *half_patterns[2 * b])
                emit_mm1_exp(b, 1, *half_patterns[2 * b + 1])
            else:
                emit_mm1_exp(b, 0, *half_patterns[6])
                emit_mm1_exp(b, 1, ((4, 3, 4, 3, 2)), ((0, 1, 0, 1, 0)))
            if b >= 1:
                p = b - 1
                for h in range(2):
                    g = grams[(p, h)]
                    emit_mm2_j(p, 2 * h, g, 0)
                    emit_mm2_j(p, 2 * h + 1, g, 8)
                    emit_norm(p, slice(8 * h, 8 * h + 8), nc.sync)
        b = B - 1
        for h in range(2):
            g = grams[(b, h)]
            emit_mm2_j(b, 2 * h, g, 0)
            emit_mm2_j(b, 2 * h + 1, g, 8)
            emit_norm(b, slice(8 * h, 8 * h + 8), nc.sync)

    nc.compile()
    return nc


def _split_hi_lo(a):
    import ml_dtypes

    bf = ml_dtypes.bfloat16
    hi = a.astype(bf).astype(np.float32)
    lo = (a - hi).astype(bf).astype(np.float32)
    return hi, lo


def _prepare_inputs(X, Y, grid):
    import ml_dtypes

    bf = ml_dtypes.bfloat16
    X = np.asarray(X, np.float32)
    Y = np.asarray(Y, np.float32)
    grid = np.asarray(grid, np.float32)

    sx = -0.5 * np.sum(X * X, axis=-1)
    sg = -0.5 * np.sum(grid * grid, axis=-1)
    xh, xl = _split_hi_lo(X)
    gh, gl = _split_hi_lo(grid)
    sxh, sxl = _split_hi_lo(sx)
    sgh, sgl = _split_hi_lo(sg)
    ones_n = np.ones((B, N), np.float32)
    ones_g = np.ones((G,), np.float32)

    A = np.stack(
        [xh[..., 0], xh[..., 1], xl[..., 0], xl[..., 1],
         xh[..., 0], xh[..., 1], sxh, sxl, ones_n, ones_n],
        axis=1,
    )
    Bm = np.stack(
        [gh[:, 0], gh[:, 1], gh[:, 0], gh[:, 1],
         gl[:, 0], gl[:, 1], ones_g, ones_g, sgh, sgl],
        axis=0,
    )

    A4 = A.transpose(1, 0, 2).reshape(K, B, 2, 4, 128)
    arep = np.zeros((128, B, 2, 128), np.float32)
    for i in range(4):
        arep[32 * i:32 * i + K] = A4[:, :, :, i, :]

    yh, yl = _split_hi_lo(Y)
    E = np.stack([ones_n, yh[..., 0], yh[..., 1], yl[..., 0], yl[..., 1]],
                 axis=-1)
    ey = E.reshape(B, NTG, 128, 5).transpose(2, 0, 1, 3)

    in_maps = []
    ar = arep.reshape(128, A_W)
    for c in range(NCORES):
        brep = np.zeros((128, GS), np.float32)
        for i in range(4):
            brep[32 * i:32 * i + K] = Bm[:, c * GS:(c + 1) * GS]
        packed = np.concatenate(
            [ar[:, 0:256], brep[:, 0:512], brep[:, 512:1024],
             ar[:, 256:A_W], brep[:, 1024:1536], brep[:, 1536:2048],
             ey.reshape(128, E_W)], axis=1)
        in_maps.append({"IN": np.ascontiguousarray(packed).astype(bf)})
    return in_maps


def _run(in_maps, builder, key, trace=False):
    from concourse.bass_utils import run_bass_kernel_spmd

    if key not in _CACHE:
        _CACHE[key] = builder()
    nc = _CACHE[key]
    return run_bass_kernel_spmd(nc, in_maps, core_ids=list(range(NCORES)),
                                trace=trace)


def kernel(X, Y, grid, _trace=False, _results_out=None):
    out = np.empty((B, 3, 128, 128), np.float32)
    if _grid_separable(grid):
        in_maps = _prepare_inputs_v2(X, Y, grid)
        res = _run(in_maps, _build_nc_v2, "v2", trace=_trace)
        for c in range(NCORES):
            b, h = divmod(c, 2)
            o = res.results[c]["OUT"].astype(np.float32)   # (128x, 3, 64y)
            out[b, :, 64 * h:64 * h + 64, :] = o.transpose(1, 2, 0)
    else:
        in_maps = _prepare_inputs(X, Y, grid)
        res = _run(in_maps, _build_nc_general, "gen", trace=_trace)
        for c in range(NCORES):
            out[:, :, :, c * XCOLS:(c + 1) * XCOLS] = res.results[c]["OUT"]
    if _results_out is not None:
        _results_out.append(res)
    return out


# revision 15
# speedup vs baseline: 1.0130x; 1.0130x over previous
"""ConvCNP encoder kernel for 8 Trainium2 NeuronCores.

Computes, for full inputs X(4,1024,2), Y(4,1024,2), grid(16384,2):
    Gram = exp(-0.5*||grid-X||^2)          (B, G, n)
    FM   = Gram @ [1, Y]                   (B, G, 3)
    out  = [FM0, FM1/FM0, FM2/FM0] -> (B, 3, 128, 128)  (y, x image axes)

The reference grid is a meshgrid, so the 2-D RBF factors into 1-D
Gaussians: Gram[(x,y), n] = A1[x, n] * A2[y, n].

Sharding (v2): core = (batch b, y-half h).  Each core computes its
batch over y in [64h, 64h+64) and ALL 128 x columns:
  exp work/core = N*(64+128) elems vs N*B*(128+16) for a grid-shard.

Per-core pipeline (2 chunks of 4 n-tiles):
  mm1 (PE):  A2pre[n,y] = s2[7,n].T @ g2[7,64],  A1pre[n,x] likewise
  exp (ACT): PSUM -> SBUF A2/A1 (f16)
  V   (DVE): V[n,c,y] = A2[n,y] * E[n,c]  (broadcast mul)
  mm2 (PE):  FM[x,(c,y)] += A1[n,x].T @ V[n,(c,y)]  over 8 n-tiles
  norm:      recip + muls, density copy on ACT
  DMA out:   one [128, 768B] contiguous store; host reassembles.
"""

import numpy as np

B = 4
N = 1024
G = 16384
NCORES = 8
NT = N // 128             # 8 context tiles
KS = 7                    # 1-D factorization rows
GS = G // NCORES
XCOLS = GS // 128

_CACHE = {}


# ---------------------------------------------------------------------------
# v2 separable kernel: core = (batch, y-half)
# ---------------------------------------------------------------------------

def _build_nc_v2():
    import concourse.bacc as bacc
    import concourse.mybir as mybir
    import concourse.tile as tile
    from contextlib import ExitStack

    f32 = mybir.dt.float32
    f16 = mybir.dt.float16

    nc = bacc.Bacc("TRN2", target_bir_lowering=False, debug=False,
                   num_devices=NCORES)
    # column layout A: [G2 64 | G1 128 | s2 nt0..3 512 | s1 nt0..3 512]
    # column layout B: [s2 nt4..7 512 | s1 nt4..7 512]
    insa_d = nc.dram_tensor("INSA", [KS, 1216], f16, kind="ExternalInput")
    insb_d = nc.dram_tensor("INSB", [KS, 1024], f16, kind="ExternalInput")
    # E for channels 1,2 pre-broadcast along y: [128, nt, 2, 64]
    ine_d = nc.dram_tensor("INE", [128, NT * 2 * 64], f16, kind="ExternalInput")
    out_d = nc.dram_tensor("OUT", [128, 3, 64], f16, kind="ExternalOutput")

    EXP = mybir.ActivationFunctionType.Exp

    CH = (4, 4)               # n-tile chunk sizes
    C0 = CH[0]

    with tile.TileContext(nc) as tc, ExitStack() as ctx:
        consts = ctx.enter_context(tc.tile_pool(name="consts", bufs=1))
        ps2_pool = ctx.enter_context(tc.tile_pool(name="ps2", bufs=1, space="PSUM"))
        ps1_pool = ctx.enter_context(tc.tile_pool(name="ps1", bufs=1, space="PSUM"))
        fm0_pool = ctx.enter_context(tc.tile_pool(name="fm0", bufs=1, space="PSUM"))
        fm12_pool = ctx.enter_context(tc.tile_pool(name="fm12", bufs=1, space="PSUM"))
        a2_pool = ctx.enter_context(tc.tile_pool(name="a2", bufs=1))
        a1_pool = ctx.enter_context(tc.tile_pool(name="a1", bufs=1))
        v_pool = ctx.enter_context(tc.tile_pool(name="v", bufs=1))
        small = ctx.enter_context(tc.tile_pool(name="small", bufs=2))
        outp = ctx.enter_context(tc.tile_pool(name="outp", bufs=1))

        insa = consts.tile([KS, 1216], f16)
        insb = consts.tile([KS, 1024], f16)
        e_sb = consts.tile([128, NT * 2 * 64], f16)
        # first-needed columns on sync; INSB via scalar's HWDGE; fat E
        # split over sync+gpsimd queues
        nc.sync.dma_start(out=insa, in_=insa_d[:, :])
        nc.scalar.dma_start(out=insb, in_=insb_d[:, :])
        nc.sync.dma_start(out=e_sb[:, 0:C0 * 128], in_=ine_d[:, 0:C0 * 128])
        nc.gpsimd.dma_start(out=e_sb[:, C0 * 128:], in_=ine_d[:, C0 * 128:])

        e_v = e_sb.rearrange("p (t c y) -> p t c y", t=NT, c=2)
        g2 = insa[:, 0:64]
        g1 = insa[:, 64:192]

        def s2_sl(nt):
            t, off = (insa, 192) if nt < 4 else (insb, -512)
            return t[:, off + 128 * nt: off + 128 * (nt + 1)]

        def s1_sl(nt):
            t, off = (insa, 704) if nt < 4 else (insb, 0)
            return t[:, off + 128 * nt: off + 128 * (nt + 1)]

        out_sb = outp.tile([128, 3, 64], f16)
        fm0 = fm0_pool.tile([128, 64], f32)
        fm12 = fm12_pool.tile([128, 2, 64], f32)

        a2t, a1t, vt = {}, {}, {}
        # per chunk: A2 matmuls, A1 matmuls, exp(A2), exp(A1), V — this
        # interleaving lets mm2 (gated on a1 of chunk 0) start early
        for ch in range(2):
            n0 = ch * C0
            ps2 = ps2_pool.tile([128, CH[ch], 64], f32, tag=f"ps2{ch}",
                                name=f"ps2_{ch}")
            for i in range(CH[ch]):
                nc.tensor.matmul(ps2[:, i, :], s2_sl(n0 + i), g2,
                                 start=True, stop=True)
            ps1 = ps1_pool.tile([128, CH[ch], 128], f32, tag=f"ps1{ch}",
                                name=f"ps1_{ch}")
            for i in range(CH[ch]):
                nc.tensor.matmul(ps1[:, i, :], s1_sl(n0 + i), g1,
                                 start=True, stop=True)
            a2 = a2_pool.tile([128, CH[ch], 64], f16, tag=f"a2{ch}",
                              name=f"a2_{ch}")
            nc.scalar.activation(out=a2, in_=ps2, func=EXP)
            a2t[ch] = a2
            a1 = a1_pool.tile([128, CH[ch], 128], f16, tag=f"a1{ch}",
                              name=f"a1_{ch}")
            nc.scalar.activation(out=a1, in_=ps1, func=EXP)
            a1t[ch] = a1
            v = v_pool.tile([128, CH[ch], 2, 64], f16, tag=f"v{ch}",
                            name=f"v_{ch}")
            a2_b = a2[:, :, None, :].to_broadcast([128, CH[ch], 2, 64])
            nc.vector.tensor_mul(v, a2_b, e_v[:, n0:n0 + CH[ch], :, :])
            vt[ch] = v

        # mm2, chunk-major so chunk 1's groups are all that trail the last
        # exp; fm0 (density) closes before fm12 so recip/copy overlap
        for ch in range(2):
            for i in range(CH[ch]):
                nc.tensor.matmul(fm0[:], a1t[ch][:, i, :], a2t[ch][:, i, :],
                                 start=(ch == 0 and i == 0),
                                 stop=(ch == 1 and i == CH[1] - 1))
            for i in range(CH[ch]):
                nc.tensor.matmul(fm12[:], a1t[ch][:, i, :], vt[ch][:, i, :, :],
                                 start=(ch == 0 and i == 0),
                                 stop=(ch == 1 and i == CH[1] - 1))

        recip = small.tile([128, 64], f32, tag="recip")
        nc.vector.reciprocal_approx_fast(out=recip, in_=fm0[:])
        nc.scalar.copy(out_sb[:, 0, :], fm0[:])
        rec_b = recip[:, None, :].to_broadcast([128, 2, 64])
        nc.vector.tensor_mul(out_sb[:, 1:3, :], fm12[:], rec_b)
        nc.sync.dma_start(out=out_d[:, :, :], in_=out_sb)

    nc.compile()
    return nc


def _sep_factors(gv, xc):
    """K=7 fp16 factorization of -0.5 (g - x)^2 along one dimension.
    xc: (N,) context coords, gv: (M,) grid coords.
    Returns A [7, N] stationary rows and Bm [7, M] moving rows."""
    f16 = np.float16

    def split(a):
        hi = a.astype(f16).astype(np.float32)
        lo = (a - hi).astype(f16).astype(np.float32)
        return hi, lo

    sx = -0.5 * xc * xc
    sg = -0.5 * gv * gv
    xh, xl = split(xc)
    gh, gl = split(gv)
    sxh, sxl = split(sx)
    sgh, sgl = split(sg)
    on = np.ones_like(xc)
    og = np.ones_like(gv)
    A = np.stack([xh, xl, xh, sxh, sxl, on, on], axis=0)
    Bm = np.stack([gh, gh, gl, og, og, sgh, sgl], axis=0)
    return A.astype(f16), Bm.astype(f16)


def _prepare_inputs_v2(X, Y, grid):
    f16 = np.float16
    X = np.asarray(X, np.float32)
    Y = np.asarray(Y, np.float32)
    grid = np.asarray(grid, np.float32)
    gxv = grid[::128, 0]
    gyv = grid[:128, 1]

    in_maps = []
    for c in range(NCORES):
        b, h = divmod(c, 2)
        A1s, G1 = _sep_factors(gxv, X[b, :, 0])                 # [7,1024],[7,128]
        A2s, G2 = _sep_factors(gyv[64 * h:64 * h + 64], X[b, :, 1])  # [7,1024],[7,64]
        s1 = A1s.reshape(KS, NT, 128)
        s2 = A2s.reshape(KS, NT, 128)
        insa = np.concatenate(
            [G2, G1, s2[:, 0:4].reshape(KS, 512), s1[:, 0:4].reshape(KS, 512)],
            axis=1)
        insb = np.concatenate(
            [s2[:, 4:8].reshape(KS, 512), s1[:, 4:8].reshape(KS, 512)], axis=1)
        # E channels 1,2 pre-broadcast along y: [128, nt, 2, 64]
        e12 = Y[b].reshape(NT, 128, 2).transpose(1, 0, 2).astype(f16)
        ine = np.broadcast_to(e12[:, :, :, None],
                              (128, NT, 2, 64)).reshape(128, NT * 2 * 64)
        in_maps.append({"INSA": np.ascontiguousarray(insa).astype(f16),
                        "INSB": np.ascontiguousarray(insb).astype(f16),
                        "INE": np.ascontiguousarray(ine).astype(f16)})
    return in_maps


def _grid_separable(grid):
    grid = np.asarray(grid)
    if grid.shape != (G, 2):
        return False
    gxv = grid[::128, 0]
    gyv = grid[:128, 1]
    return (np.array_equal(grid[:, 0], np.repeat(gxv, 128))
            and np.array_equal(grid[:, 1], np.tile(gyv, 128)))


# ---------------------------------------------------------------------------
# General fallback (grid not a meshgrid): grid-axis shard, K=10 bf16
# hi/lo-split factorization of -0.5*d^2.  Unused for the reference grid.
# ---------------------------------------------------------------------------

NTG = N // 128
JS = GS // 512
K = 10
A_W = B * 2 * 128
B_W = GS
E_W = B * NTG * 5
IN_W = A_W + B_W + E_W


def _build_nc_general():
    import concourse.bacc as bacc
    import concourse.mybir as mybir
    import concourse.tile as tile
    from contextlib import ExitStack

    f32 = mybir.dt.float32
    bf16 = mybir.dt.bfloat16

    nc = bacc.Bacc("TRN2", target_bir_lowering=False, debug=False,
                   num_devices=NCORES)
    in_d = nc.dram_tensor("IN", [128, IN_W], bf16, kind="ExternalInput")
    out_d = nc.dram_tensor("OUT", [B, 3, 128, XCOLS], f32, kind="ExternalOutput")

    EXP = mybir.ActivationFunctionType.Exp

    with tile.TileContext(nc) as tc, ExitStack() as ctx:
        consts = ctx.enter_context(tc.tile_pool(name="consts", bufs=1))
        gram_pool = ctx.enter_context(tc.tile_pool(name="gram", bufs=4))
        mm1a_pool = ctx.enter_context(tc.tile_pool(name="mm1a", bufs=1, space="PSUM"))
        mm1b_pool = ctx.enter_context(tc.tile_pool(name="mm1b", bufs=1, space="PSUM"))
        mm2_pool = ctx.enter_context(tc.tile_pool(name="mm2", bufs=1, space="PSUM"))
        small = ctx.enter_context(tc.tile_pool(name="small", bufs=4))
        outp = ctx.enter_context(tc.tile_pool(name="outp", bufs=1))

        a0_sb = consts.tile([128, 2 * 128], bf16)
        a123_sb = consts.tile([128, 3 * 2 * 128], bf16)
        b_t = [consts.tile([128, 512], bf16, name=f"bj{j}", tag=f"bj{j}")
               for j in range(JS)]
        e_sb = consts.tile([128, E_W], bf16)

        def in_col(c0, w):
            return in_d[:, c0:c0 + w]

        nc.sync.dma_start(out=a0_sb, in_=in_col(0, 256))
        nc.sync.dma_start(out=b_t[0], in_=in_col(256, 512))
        nc.gpsimd.dma_start(out=b_t[1], in_=in_col(768, 512))
        nc.sync.dma_start(out=a123_sb, in_=in_col(1280, 768))
        nc.gpsimd.dma_start(out=b_t[2], in_=in_col(2048, 512))
        nc.sync.dma_start(out=b_t[3], in_=in_col(2560, 512))
        nc.gpsimd.dma_start(out=e_sb, in_=in_col(3072, E_W))

        a0_v = a0_sb.rearrange("p (h m) -> p h m", h=2)
        a123_v = a123_sb.rearrange("p (b h m) -> p b h m", b=3, h=2)
        e_v = e_sb.rearrange("p (b t c) -> p b t c", b=B, t=NTG)

        def a_slice(b, row, h4):
            if b == 0:
                return a0_v[32 * row:32 * row + K, h4, :]
            return a123_v[32 * row:32 * row + K, b - 1, h4, :]

        def b_slice(j):
            return b_t[j]

        out_sb = outp.tile([128, B, 3, XCOLS], f32)
        grams = {}

        def emit_mm1_group(b, slots, ps):
            for i, s in enumerate(slots):
                j = s // 8
                nt = s % 8
                row = nt % 4
                lhsT = a_slice(b, row, nt // 4)
                rhs = b_slice(j)[32 * row:32 * row + K, :]
                nc.tensor.matmul(ps[:, i, :], lhsT, rhs,
                                 start=True, stop=True,
                                 tile_position=(32 * row, 0))

        def emit_mm1_exp(b, h, sizes, tags):
            gram = gram_pool.tile([128, 16, 512], bf16, tag="gram",
                                  name=f"gram{b}{h}")
            grams[(b, h)] = gram
            s0 = 0
            for gsz, sel in zip(sizes, tags):
                pool = (mm1a_pool, mm1b_pool)[sel]
                cap = (4, 3)[sel]
                ps = pool.tile([128, cap, 512], f32, tag=f"t{sel}",
                               name=f"ps{sel}")
                emit_mm1_group(b, [16 * h + s0 + i for i in range(gsz)], ps)
                nc.scalar.activation(out=gram[:, s0:s0 + gsz, :],
                                     in_=ps[:, 0:gsz, :], func=EXP)
                s0 += gsz

        def emit_mm2_j(b, j, gram, base):
            fm = grams[("fm", b)]
            for r in range(4):
                gsub = j * 4 + r
                for nt in range(NTG):
                    nc.tensor.matmul(
                        fm[:, gsub, :],
                        gram[:, base + nt, r * 128:(r + 1) * 128],
                        e_v[:, b, nt, :],
                        start=(nt == 0),
                        stop=(nt == NTG - 1),
                    )

        def emit_norm(b, sl, dma_engine):
            fm = grams[("fm", b)]
            w = sl.stop - sl.start
            fmc = small.tile([128, 8, 5], f32, tag="fmc")
            nc.vector.tensor_copy(fmc[:, 0:w, :], fm[:, sl, :])
            recip = small.tile([128, 8], f32, tag="recip")
            nc.vector.reciprocal(recip[:, 0:w], fmc[:, 0:w, 0])
            nc.vector.tensor_copy(out_sb[:, b, 0, sl], fmc[:, 0:w, 0])
            v1 = small.tile([128, 8], f32, tag="v1")
            nc.vector.tensor_add(v1[:, 0:w], fmc[:, 0:w, 1], fmc[:, 0:w, 3])
            nc.vector.tensor_mul(out_sb[:, b, 1, sl], v1[:, 0:w], recip[:, 0:w])
            v2 = small.tile([128, 8], f32, tag="v2")
            nc.vector.tensor_add(v2[:, 0:w], fmc[:, 0:w, 2], fmc[:, 0:w, 4])
            nc.vector.tensor_mul(out_sb[:, b, 2, sl], v2[:, 0:w], recip[:, 0:w])
            dst = out_d[b, :, :, sl].rearrange("c y x -> y c x")
            dma_engine.dma_start(out=dst, in_=out_sb[:, b, :, sl])

        half_patterns = [((2, 4, 3, 4, 3), (1, 0, 1, 0, 1))]
        for k in range(1, 7):
            if k % 2 == 1:
                half_patterns.append(((4, 3, 4, 3, 2), (0, 1, 0, 1, 0)))
            else:
                half_patterns.append(((3, 4, 3, 4, 2), (1, 0, 1, 0, 1)))

        for b in range(B):
            fm_t = mm2_pool.tile([128, XCOLS, 5], f32, tag="fm")
            grams[("fm", b)] = fm_t
            if b < B - 1:
                emit_mm1_exp(b, 0, *half_patterns[2 * b])
                emit_mm1_exp(b, 1, *half_patterns[2 * b + 1])
            else:
                emit_mm1_exp(b, 0, *half_patterns[6])
                emit_mm1_exp(b, 1, ((4, 3, 4, 3, 2)), ((0, 1, 0, 1, 0)))
            if b >= 1:
                p = b - 1
                for h in range(2):
                    g = grams[(p, h)]
                    emit_mm2_j(p, 2 * h, g, 0)
                    emit_mm2_j(p, 2 * h + 1, g, 8)
                    emit_norm(p, slice(8 * h, 8 * h + 8), nc.sync)
        b = B - 1
        for h in range(2):
            g = grams[(b, h)]
            emit_mm2_j(b, 2 * h, g, 0)
            emit_mm2_j(b, 2 * h + 1, g, 8)
            emit_norm(b, slice(8 * h, 8 * h + 8), nc.sync)

    nc.compile()
    return nc


def _split_hi_lo(a):
    import ml_dtypes

    bf = ml_dtypes.bfloat16
    hi = a.astype(bf).astype(np.float32)
    lo = (a - hi).astype(bf).astype(np.float32)
    return hi, lo


def _prepare_inputs(X, Y, grid):
    import ml_dtypes

    bf = ml_dtypes.bfloat16
    X = np.asarray(X, np.float32)
    Y = np.asarray(Y, np.float32)
    grid = np.asarray(grid, np.float32)

    sx = -0.5 * np.sum(X * X, axis=-1)
    sg = -0.5 * np.sum(grid * grid, axis=-1)
    xh, xl = _split_hi_lo(X)
    gh, gl = _split_hi_lo(grid)
    sxh, sxl = _split_hi_lo(sx)
    sgh, sgl = _split_hi_lo(sg)
    ones_n = np.ones((B, N), np.float32)
    ones_g = np.ones((G,), np.float32)

    A = np.stack(
        [xh[..., 0], xh[..., 1], xl[..., 0], xl[..., 1],
         xh[..., 0], xh[..., 1], sxh, sxl, ones_n, ones_n],
        axis=1,
    )
    Bm = np.stack(
        [gh[:, 0], gh[:, 1], gh[:, 0], gh[:, 1],
         gl[:, 0], gl[:, 1], ones_g, ones_g, sgh, sgl],
        axis=0,
    )

    A4 = A.transpose(1, 0, 2).reshape(K, B, 2, 4, 128)
    arep = np.zeros((128, B, 2, 128), np.float32)
    for i in range(4):
        arep[32 * i:32 * i + K] = A4[:, :, :, i, :]

    yh, yl = _split_hi_lo(Y)
    E = np.stack([ones_n, yh[..., 0], yh[..., 1], yl[..., 0], yl[..., 1]],
                 axis=-1)
    ey = E.reshape(B, NTG, 128, 5).transpose(2, 0, 1, 3)

    in_maps = []
    ar = arep.reshape(128, A_W)
    for c in range(NCORES):
        brep = np.zeros((128, GS), np.float32)
        for i in range(4):
            brep[32 * i:32 * i + K] = Bm[:, c * GS:(c + 1) * GS]
        packed = np.concatenate(
            [ar[:, 0:256], brep[:, 0:512], brep[:, 512:1024],
             ar[:, 256:A_W], brep[:, 1024:1536], brep[:, 1536:2048],
             ey.reshape(128, E_W)], axis=1)
        in_maps.append({"IN": np.ascontiguousarray(packed).astype(bf)})
    return in_maps


def _run(in_maps, builder, key, trace=False):
    from concourse.bass_utils import run_bass_kernel_spmd

    if key not in _CACHE:
        _CACHE[key] = builder()
    nc = _CACHE[key]
    return run_bass_kernel_spmd(nc, in_maps, core_ids=list(range(NCORES)),
                                trace=trace)


def kernel(X, Y, grid, _trace=False, _results_out=None):
    out = np.empty((B, 3, 128, 128), np.float32)
    if _grid_separable(grid):
        in_maps = _prepare_inputs_v2(X, Y, grid)
        res = _run(in_maps, _build_nc_v2, "v2", trace=_trace)
        for c in range(NCORES):
            b, h = divmod(c, 2)
            o = res.results[c]["OUT"].astype(np.float32)   # (128x, 3, 64y)
            out[b, :, 64 * h:64 * h + 64, :] = o.transpose(1, 2, 0)
    else:
        in_maps = _prepare_inputs(X, Y, grid)
        res = _run(in_maps, _build_nc_general, "gen", trace=_trace)
        for c in range(NCORES):
            out[:, :, :, c * XCOLS:(c + 1) * XCOLS] = res.results[c]["OUT"]
    if _results_out is not None:
        _results_out.append(res)
    return out
